# revision 26
# baseline (speedup 1.0000x reference)
"""Single-query attention pooling + linear head, sharded batch-parallel
across 8 Trainium2 NeuronCores.  fp16 redesign of the fp32 baseline.

Reference computation (per batch b):
    score[s]  = sum_h inp[b,s,h] * q[b,h]
    score    -= 1e30 * (1 - mask)                (additive mask)
    att       = softmax(score)
    ext[b,h]  = sum_s att[s] * inp[b,s,h]
    ctrl[b,:] = W @ concat(q[b], ext[b]) + bias

Key changes vs the fp32 baseline (219 us -> 123.5 us measured):
  - inp/q/W cast to fp16 on the HOST: HBM traffic per core drops from
    33.5 MB to ~18 MB, and fp16 enables the DVE 2x tensor_tensor mode
    for the score products (4 instrs/batch, stride-0 broadcast q AP).
  - score reduction split ACT (copy + fused accumulator, 12 chunks) /
    DVE (binary fold tree, 20 chunks; last two levels fp32 since the
    largest partial sums dominate the fp16 rounding error).
  - 1/denominator via nc.vector.reciprocal: the old Ln+Exp Newton chain
    paid two ~1.3us ACT_TABLE_LOADs per batch (Ln and Exp live in
    different LUT sets).
  - masked row max via a single tensor_reduce instead of a 5-op tree.
  - host pre-broadcasts q ([128,B,256] fp16) so no on-chip broadcast.
  - numerator via 32 accumulating fp16 PE matmuls (moving 2 cols/cyc).
  - linear head on the PE: per-batch ext is PE-transposed into a column
    tile; one batched 8-matmul pass against host-prepped W^T blocks at
    the end replaces per-batch DVE/ACT head work.
  - mask preprocessed on host to the additive form (-1e30*(1-mask)).
  - GpSimd does only the two partition all-reduces (max, sum); a
    GpSimd-fold variant measured 1.67x slower (slow strict-FIFO Q7 ops
    on every batch's softmax critical path).

Measured (8-core HW): 123.5 us, ACT 71.5us / DVE 70.9us busy per core
(balanced), rel err 7.4e-3 absmax vs the fp32 reference (gate 2e-2).
"""

import numpy as np
from contextlib import ExitStack

import concourse.bacc as bacc
import concourse.mybir as mybir
import concourse.tile as tile
from concourse import bass_isa, bass_utils

P = 128          # SBUF partitions
C = 32           # seq chunks; position s = p*C + c
S = P * C        # 4096
H = 256
H2 = 2 * H
N_CORES = 8
B_TOTAL = 64
B = B_TOTAL // N_CORES   # batches per core

# score-reduction split (chunks per batch): ACT copy+accum vs DVE fold.
# (A GpSimd-fold variant measured 1.67x SLOWER: the Q7 engine's slow
# strict-FIFO tensor ops landed on every batch's softmax critical path.)
R_ACT = 12
R_DVE = C - R_ACT
R_GPS = 0

F32 = mybir.dt.float32
F16 = mybir.dt.float16
AF = mybir.ActivationFunctionType
ALU = mybir.AluOpType

_CACHE = {}


def _fold_tree(eng, prod, scf, sc, c0, k):
    """Binary-fold sum over the last dim of prod[:, c0:c0+k, 0:256] (fp16).
    Levels down to width 8 stay fp16 in-place; the last levels (largest
    partial-sum magnitudes, so largest fp16 rounding) run in fp32 via the
    scf scratch.  Final level writes sc[:, c0:c0+k] (fp32)."""
    w = H // 2
    while w > 4:
        eng.tensor_tensor(
            out=prod[:, c0 : c0 + k, 0:w],
            in0=prod[:, c0 : c0 + k, 0:w],
            in1=prod[:, c0 : c0 + k, w : 2 * w],
            op=ALU.add,
        )
        w //= 2
    # w == 4: fp16 in -> fp32 out
    eng.tensor_tensor(
        out=scf[:, 0:k, :],
        in0=prod[:, c0 : c0 + k, 0:4],
        in1=prod[:, c0 : c0 + k, 4:8],
        op=ALU.add,
    )
    eng.tensor_tensor(
        out=scf[:, 0:k, 0:2],
        in0=scf[:, 0:k, 0:2],
        in1=scf[:, 0:k, 2:4],
        op=ALU.add,
    )
    eng.tensor_tensor(
        out=sc[:, c0 : c0 + k],
        in0=scf[:, 0:k, 0],
        in1=scf[:, 0:k, 1],
        op=ALU.add,
    )


def build_nc():
    nc = bacc.Bacc("TRN2", target_bir_lowering=False)

    inp = nc.dram_tensor("inp", [B, S, H], F16, kind="ExternalInput")
    qb1 = nc.dram_tensor("qb1", [P, B, H], F16, kind="ExternalInput")
    qcol = nc.dram_tensor("qcol", [P, 2, B], F16, kind="ExternalInput")
    wcol = nc.dram_tensor("wcol", [P, 4, 2, P], F16, kind="ExternalInput")
    bcol = nc.dram_tensor("bcol", [P, 2], F32, kind="ExternalInput")
    madd = nc.dram_tensor("madd", [P, B, C], F32, kind="ExternalInput")
    ext = nc.dram_tensor("ext", [B, H], F32, kind="ExternalOutput")
    ctl = nc.dram_tensor("ctl", [B, H], F32, kind="ExternalOutput")

    with ExitStack() as ctx:
        tc = ctx.enter_context(tile.TileContext(nc))
        const = ctx.enter_context(tc.tile_pool(name="const", bufs=1))
        inpp = ctx.enter_context(tc.tile_pool(name="inpp", bufs=4))
        prdp = ctx.enter_context(tc.tile_pool(name="prdp", bufs=2))
        smal = ctx.enter_context(tc.tile_pool(name="smal", bufs=4))
        scr = ctx.enter_context(tc.tile_pool(name="scr", bufs=2))
        psum = ctx.enter_context(tc.tile_pool(name="psum", bufs=2, space="PSUM"))
        psum1 = ctx.enter_context(tc.tile_pool(name="psum1", bufs=1, space="PSUM"))

        # --- one-time loads (replicated weights + all-batch small inputs) ---
        # heavy consts ride the ACT HWDGE ring so batch 0's inp load (SP
        # ring) isn't stuck behind them in FIFO order
        qb_sb = const.tile([P, B, H], F16)
        nc.scalar.dma_start(qb_sb[:], qb1[:])
        qc_sb = const.tile([P, 2, B], F16)
        nc.scalar.dma_start(qc_sb[:], qcol[:])
        wc_sb = const.tile([P, 4, 2, P], F16)
        nc.scalar.dma_start(wc_sb[:], wcol[:])
        bc_sb = const.tile([P, 2], F32)
        nc.scalar.dma_start(bc_sb[:], bcol[:])
        m1 = const.tile([P, B, C], F32)
        nc.scalar.dma_start(m1[:], madd[:])
        id1 = const.tile([1, 1], F32)
        nc.vector.memset(id1[:], 1.0)

        # head column accumulators (per-batch transposed ext lands here)
        excol_ps = psum1.tile([P, 2, B], F32, tag="excol")
        conc = const.tile([P, 4, B], F16)
        nc.vector.tensor_copy(conc[:, 0:2, :], qc_sb[:])

        for b in range(B):
            # --- stage inp[b]: [4096, 256] -> [128, 32, 256], s = p*32+c ---
            it = inpp.tile([P, C, H], F16, tag="it")
            src = inp[b].rearrange("(p c) h -> p c h", p=P)
            if b == 0:
                # batch 0: 4 pieces so products start before the full 2MB lands
                for j in range(0, C, 8):
                    nc.sync.dma_start(it[:, j : j + 8, :], src[:, j : j + 8, :])
            else:
                nc.sync.dma_start(it[:], src)

            # --- score products (DVE fp16 2x, broadcast q along chunks) ---
            qb_b = qb_sb[:, b, :].unsqueeze(1).broadcast_to((P, 8, H))
            prod = prdp.tile([P, C, H], F16, tag="prod")
            for j in range(0, C, 8):
                nc.vector.tensor_tensor(
                    out=prod[:, j : j + 8, :],
                    in0=it[:, j : j + 8, :],
                    in1=qb_b,
                    op=ALU.mult,
                )

            # --- score reductions: ACT chunks [0,R_ACT), DVE fold the rest ---
            sc = smal.tile([P, C], F32, tag="sc")
            dmp = scr.tile([P, H], F16, tag="dmp")
            for j in range(R_ACT):
                nc.scalar.activation(
                    dmp[:], prod[:, j, :], AF.Copy, accum_out=sc[:, j : j + 1]
                )
            scf = scr.tile([P, R_DVE, 4], F32, tag="scf")
            _fold_tree(nc.vector, prod, scf, sc, R_ACT, R_DVE)
            if R_GPS:
                scg = scr.tile([P, R_GPS, 4], F32, tag="scg")
                _fold_tree(nc.gpsimd, prod, scg, sc, R_ACT + R_DVE, R_GPS)

            # --- additive mask + masked global max ---
            nc.vector.tensor_tensor(out=sc[:], in0=sc[:], in1=m1[:, b, :], op=ALU.add)
            mx = smal.tile([P, 1], F32, tag="mx")
            nc.vector.tensor_reduce(mx[:], sc[:], mybir.AxisListType.X, ALU.max)
            gmaxb = smal.tile([P, 1], F32, tag="gmaxb")
            nc.gpsimd.partition_all_reduce(
                gmaxb[:], mx[:], channels=P, reduce_op=bass_isa.ReduceOp.max
            )
            nmaxb = smal.tile([P, 1], F32, tag="nmaxb")
            nc.vector.tensor_scalar_mul(nmaxb[:], gmaxb[:], -1.0)
            # clamp at (gmax - 88) so the exp input range stays LUT-safe
            clampb = smal.tile([P, 1], F32, tag="clampb")
            nc.vector.tensor_scalar_add(clampb[:], gmaxb[:], -88.0)
            nc.vector.tensor_scalar_max(sc[:], sc[:], clampb[:])

            # --- softmax weights (fp16 for the PE) + denominator ---
            wgt = smal.tile([P, C], F16, tag="wgt")
            part = smal.tile([P, 1], F32, tag="part")
            nc.scalar.activation(
                wgt[:], sc[:], AF.Exp, bias=nmaxb[:], scale=1.0, accum_out=part[:]
            )
            denb = smal.tile([P, 1], F32, tag="denb")
            nc.gpsimd.partition_all_reduce(
                denb[:], part[:], channels=P, reduce_op=bass_isa.ReduceOp.add
            )
            # 1/den on the DVE divide unit (avoids the ACT Ln/Exp table
            # thrash: Ln and Exp live in different LUT sets, so the old
            # chain paid two ~1.3us ACT_TABLE_LOADs per batch)
            rdenb = smal.tile([P, 1], F32, tag="rdenb")
            nc.vector.reciprocal(rdenb[:], denb[:])

            # --- numerator: [1,H] += wgt[:,c].T @ it[:,c,:] over chunks ---
            pnum = psum.tile([1, H], F32, tag="pnum")
            for c in range(C):
                nc.tensor.matmul(
                    pnum[:],
                    wgt[:, c : c + 1],
                    it[:, c, :],
                    start=(c == 0), stop=(c == C - 1),
                )

            # ext out (fp32)
            extb = smal.tile([1, H], F32, tag="extb")
            nc.scalar.activation(extb[:], pnum[:], AF.Copy, scale=rdenb[0:1, :])
            nc.sync.dma_start(ext[b : b + 1, :], extb[:])

            # transpose ext halves into the head column tile
            for g in range(2):
                nc.tensor.transpose(
                    excol_ps[:, g, b : b + 1],
                    extb[0:1, g * P : (g + 1) * P],
                    id1[:],
                )

        # --- linear head for all batches: ctl = W @ [q; ext] + bias ---
        nc.scalar.activation(conc[:, 2:4, :], excol_ps[:], AF.Copy)
        ctl_ps = psum1.tile([P, 2, B], F32, tag="ctlps")
        for ob in range(2):
            for kg in range(4):
                nc.tensor.matmul(
                    ctl_ps[:, ob, :],
                    wc_sb[:, kg, ob, :],
                    conc[:, kg, :],
                    start=(kg == 0), stop=(kg == 3),
                )
        cc = smal.tile([P, 2, B], F32, tag="cc")
        for ob in range(2):
            nc.vector.tensor_scalar_add(
                cc[:, ob, :], ctl_ps[:, ob, :], bc_sb[:, ob : ob + 1]
            )
        ctl_pgb = ctl.rearrange("b (g p) -> p g b", p=P)
        for ob in range(2):
            nc.sync.dma_start(ctl_pgb[:, ob, :], cc[:, ob, :])

    nc.compile()
    return nc


def get_nc():
    if "nc" not in _CACHE:
        _CACHE["nc"] = build_nc()
    return _CACHE["nc"]


def make_in_maps(inp_seq, mask, query, W, b):
    inp_seq = np.asarray(inp_seq, dtype=np.float32)
    mask = np.asarray(mask, dtype=np.float32)
    query = np.asarray(query, dtype=np.float32)
    W = np.asarray(W, dtype=np.float32)
    b = np.asarray(b, dtype=np.float32)

    inp16 = np.ascontiguousarray(inp_seq.astype(np.float16))
    q16 = query.astype(np.float16)
    # additive mask, rearranged to [P, B, C] with s = p*C + c
    madd_full = (-1e30 * (1.0 - mask)).astype(np.float32)
    # W^T blocks: wcol[kp, kg, ob, op] = W[ob*128+op, kg*128+kp]
    w16 = W.astype(np.float16)
    wcol = np.ascontiguousarray(
        w16.reshape(2, P, 4, P).transpose(3, 2, 0, 1)
    )
    bcol = np.ascontiguousarray(b.astype(np.float32).reshape(2, P).T)

    in_maps = []
    for i in range(N_CORES):
        lo, hi = i * B, (i + 1) * B
        qi = q16[lo:hi]                                   # [B, H]
        qb1 = np.ascontiguousarray(
            np.broadcast_to(qi[None, :, :], (P, B, H))
        )
        qcol = np.ascontiguousarray(
            qi.reshape(B, 2, P).transpose(2, 1, 0)        # [P, 2, B]
        )
        maddi = np.ascontiguousarray(
            madd_full[lo:hi].reshape(B, P, C).transpose(1, 0, 2)  # [P, B, C]
        )
        in_maps.append(
            {
                "inp": inp16[lo:hi],
                "qb1": qb1,
                "qcol": qcol,
                "wcol": wcol,
                "bcol": bcol,
                "madd": maddi,
            }
        )
    return in_maps


def assemble(results):
    ext = np.concatenate([r["ext"] for r in results], axis=0)
    ctl = np.concatenate([r["ctl"] for r in results], axis=0)
    return ext.astype(np.float32), ctl.astype(np.float32)


def kernel(inp_seq, mask, query, W, b):
    nc = get_nc()
    in_maps = make_in_maps(inp_seq, mask, query, W, b)
    res = bass_utils.run_bass_kernel_spmd(nc, in_maps, core_ids=list(range(N_CORES)))
    return assemble(res.results)
